# revision 47
# baseline (speedup 1.0000x reference)
"""Trainium2 Bass kernel for causal GQA attention (nn_Attention_83090437308676).

Full shapes: x [4096, 2048], 16 Q heads / 4 KV heads, d_head=128, fp32, causal,
rotary (interleaved pairs, rotary_dim=128), out = attn @ W_O + b_O.

Sharding: tensor-parallel over heads. Core c computes Q-heads {2c, 2c+1} and
KV-head c//2 (duplicated across the pair of cores sharing it), produces the
partial output z_h @ W_O_h summed over its 2 heads; the host sums the 8
partials (bf16 on the wire, fp32 accumulate) and adds b_O.

v4 design notes (fp32r baseline: 624us, v2 bf16: 570us, v3: 564us):
- All matmul operands are bf16 (PSUM accumulation stays fp32).
- PSUM is managed as four 4KB "big slots" ([128, 1024] fp32 = 2 banks).
  The two heads of a chunk share slots half-and-half: st2 (scores),
  zt2 (PV accumulators), den2 (softmax denominators), qp2 (Q proj),
  kv2 (K proj | V proj), op2 (out proj pairs of d_model chunks).
- One paired exp per kt tile: a single Act instruction covers both heads
  via a [128, 2, n] strided view, amortizing the PSUM access latency and
  halving Act instruction count (Act was co-critical during attention).
- Attention inner loop keeps the one-iteration skew: st matmuls of both
  heads for kt, then PV/den matmuls of kt-1. The PE never waits on exp.
- Softmax normalization is emitted in stages woven into the *next* chunk's
  projection block at points where each engine is provably past the data
  dependency: DVE reciprocals after the K rotary, PE broadcast + Act copy
  after the Q projection chains, DVE z-multiplies after the Q rotary.
- V is projected [d, k], bias-added on DVE (tensor_scalar add), and
  transposed to natural [k, d] by the PE at the end of the projection.
- Startup DMAs are just-in-time per tile (wk/xt pairs first), so the first
  kp matmul waits on ~2 DMAs; the Exp act table is preloaded via a dummy.
- Output partials are evacuated PSUM->SBUF as bf16 on the DVE in [128,1024]
  pairs and DMA'd as bf16 (2KB per partition row).
"""

import numpy as np

SEQ = 4096
D_MODEL = 2048
D_HEAD = 128
N_HEADS = 16
N_KV = 4
N_CORES = 8
ROTARY_BASE = 10000.0
ATTN_SCALE = 11.313708498984761  # sqrt(d_head)

P = 128  # partitions
FD = 512  # matmul moving free dim / PSUM bank width (fp32)
FD2 = 2 * FD


def build_bass(seq=SEQ, d_model=D_MODEL, heads_per_core=2):
    """Emit the per-core Tile kernel. Same program for all cores (SPMD);
    per-core tensors differ only in data."""
    from contextlib import ExitStack

    import concourse.mybir as mybir
    import concourse.tile as tile
    from concourse import bacc
    from concourse.bass import ds

    f32 = mybir.dt.float32
    f32r = mybir.dt.float32r
    bf16 = mybir.dt.bfloat16
    AF = mybir.ActivationFunctionType
    OP = mybir.AluOpType

    H = heads_per_core
    assert H == 2, "big-slot pairing assumes 2 heads per core"
    DM_TILES = d_model // P      # contraction tiles for projections
    QC = seq // FD               # 512-wide seq chunks
    MC = d_model // FD           # 512-wide output-model chunks
    KB = FD // P                 # 128-wide k blocks per chunk

    nc = bacc.Bacc("TRN2", target_bir_lowering=False, debug=False,
                   num_devices=N_CORES)

    xT = nc.dram_tensor("xT", (d_model, seq), bf16, kind="ExternalInput").ap()
    wq = nc.dram_tensor("wq", (H, d_model, D_HEAD), bf16, kind="ExternalInput").ap()
    wk = nc.dram_tensor("wk", (d_model, D_HEAD), bf16, kind="ExternalInput").ap()
    wv = nc.dram_tensor("wv", (d_model, D_HEAD), bf16, kind="ExternalInput").ap()
    wo = nc.dram_tensor("wo", (H, D_HEAD, d_model), bf16, kind="ExternalInput").ap()
    bq = nc.dram_tensor("bq", (64, H, 2), f32, kind="ExternalInput").ap()
    bk = nc.dram_tensor("bk", (64, 2), f32, kind="ExternalInput").ap()
    bv = nc.dram_tensor("bv", (P, 1), f32, kind="ExternalInput").ap()
    ident = nc.dram_tensor("ident", (P, P), bf16, kind="ExternalInput").ap()
    cos2 = nc.dram_tensor("cos2", (64, seq), f32, kind="ExternalInput").ap()
    sin2 = nc.dram_tensor("sin2", (64, seq), f32, kind="ExternalInput").ap()
    maskm = nc.dram_tensor("maskm", (P, P), bf16, kind="ExternalInput").ap()
    onesd = nc.dram_tensor("onesd", (P, 1), bf16, kind="ExternalInput").ap()
    onesr = nc.dram_tensor("onesr", (1, P), f32r, kind="ExternalInput").ap()
    out = nc.dram_tensor("out", (seq, d_model), bf16, kind="ExternalOutput").ap()

    with tile.TileContext(nc) as tc, ExitStack() as ctx:
        const = ctx.enter_context(tc.tile_pool(name="const", bufs=1))
        persist = ctx.enter_context(tc.tile_pool(name="persist", bufs=1))
        xt_pool = ctx.enter_context(tc.tile_pool(name="xt", bufs=32))
        qt_pool = ctx.enter_context(tc.tile_pool(name="qt", bufs=2))
        e_pool = ctx.enter_context(tc.tile_pool(name="e", bufs=4))
        sc_pool = ctx.enter_context(tc.tile_pool(name="sc", bufs=2))
        ps = ctx.enter_context(tc.tile_pool(name="ps", bufs=4, space="PSUM"))

        def big(name):
            return ps.tile([P, FD2], f32, tag="big", name=name)

        # ---- constants / weights resident in SBUF ----
        wq_sb = const.tile([P, H, DM_TILES, D_HEAD], bf16, tag="wq")
        wk_sb = const.tile([P, DM_TILES, D_HEAD], bf16, tag="wk")
        wv_sb = const.tile([P, DM_TILES, D_HEAD], bf16, tag="wv")
        wq_r = wq.rearrange("h (t p) d -> p h t d", p=P)
        wk_r = wk.rearrange("(t p) d -> p t d", p=P)
        wv_r = wv.rearrange("(t p) d -> p t d", p=P)
        mask_sb = const.tile([P, P], bf16, tag="mask")
        nc.sync.dma_start(mask_sb[:], maskm)
        bq_sb = const.tile([64, H, 2], f32, tag="bq")
        nc.sync.dma_start(bq_sb[:], bq)
        bk_sb = const.tile([64, 2], f32, tag="bk")
        nc.sync.dma_start(bk_sb[:], bk)
        bv_sb = const.tile([P, 1], f32, tag="bv")
        nc.sync.dma_start(bv_sb[:], bv)
        id_sb = const.tile([P, P], bf16, tag="id")
        nc.sync.dma_start(id_sb[:], ident)
        ones_sb = const.tile([P, 1], bf16, tag="ones")
        nc.sync.dma_start(ones_sb[:], onesd)
        onesr_sb = const.tile([1, P], f32r, tag="onesr")
        nc.sync.dma_start(onesr_sb[:], onesr)
        cos_sb = const.tile([64, seq], f32, tag="cos")
        sin_sb = const.tile([64, seq], f32, tag="sin")
        wo_sb = const.tile([P, H, d_model], bf16, tag="wo")
        # preload the Exp activation table off the critical path
        warm = const.tile([1, 2], f32, tag="warm")
        nc.scalar.activation(warm[0:1, 0:2], bq_sb[0:1, 0, 0:2], AF.Exp)

        # K^T (rotated) and V (natural [k, d]) for this core's KV head.
        kt_sb = persist.tile([P, seq], bf16, tag="kt")
        v_sb = persist.tile([P, seq // P, P], bf16, tag="v")

        xts = {}  # chunk -> list of resident xT tiles

        def prefetch_x(qc):
            tiles = [xt_pool.tile([P, FD], bf16, tag="xt", name=f"xt_{qc}_{t}")
                     for t in range(DM_TILES)]
            for t in range(DM_TILES):
                nc.sync.dma_start(tiles[t][:], xT[ds(t * P, P), ds(qc * FD, FD)])
            xts[qc] = tiles

        def rotary_evac(psum, dst, b_ap, qc):
            """dst ([P, FD] slice, bf16) = rotary(psum + bias) at positions of
            chunk qc. All DVE products run at partitions 0..63 (PSUM in0 may
            carry a different base partition; two SBUF inputs may not)."""
            sl = ds(qc * FD, FD)
            x1, x2 = psum[0:64, :], psum[64:128, :]
            b1, b2 = b_ap[:, 0:1], b_ap[:, 1:2]
            t1 = sc_pool.tile([64, FD], f32, tag="rot_t1")
            t2 = sc_pool.tile([64, FD], f32, tag="rot_t2")
            t3 = sc_pool.tile([64, FD], f32, tag="rot_t3")
            t4 = sc_pool.tile([64, FD], f32, tag="rot_t4")
            nc.vector.scalar_tensor_tensor(t1[:], x1, b1, cos_sb[:, sl],
                                           op0=OP.add, op1=OP.mult)
            nc.vector.scalar_tensor_tensor(t2[:], x2, b2, sin_sb[:, sl],
                                           op0=OP.add, op1=OP.mult)
            nc.vector.scalar_tensor_tensor(t3[:], x1, b1, sin_sb[:, sl],
                                           op0=OP.add, op1=OP.mult)
            nc.vector.scalar_tensor_tensor(t4[:], x2, b2, cos_sb[:, sl],
                                           op0=OP.add, op1=OP.mult)
            # rot1 = x1 cos - x2 sin ; rot2 = x1 sin + x2 cos
            nc.vector.tensor_sub(dst[0:64, :], t1[:], t2[:])
            nc.vector.tensor_add(dst[64:128, :], t3[:], t4[:])

        def rotary_q_pair(qp2, qt, qc):
            """Both Q heads' rotary in 6 DVE ops: bias on Act per head and
            partition-half (shifting the x2 half down to partitions 0..63 —
            two SBUF DVE inputs must share a base partition), then
            tensor-tensor products over [64, 2, FD] pair views with the
            cos/sin rows broadcast across heads via a stride-0 free dim."""
            import concourse.bass as cbass
            sl = ds(qc * FD, FD)
            qb_lo = sc_pool.tile([64, FD2], f32, tag="qb_lo", name=f"qbl_{qc}")
            qb_hi = sc_pool.tile([64, FD2], f32, tag="qb_hi", name=f"qbh_{qc}")
            for h in range(H):
                nc.scalar.activation(qb_lo[:, ds(h * FD, FD)],
                                     qp2[0:64, ds(h * FD, FD)], AF.Identity,
                                     bias=bq_sb[:, h, 0:1])
                nc.scalar.activation(qb_hi[:, ds(h * FD, FD)],
                                     qp2[64:128, ds(h * FD, FD)], AF.Identity,
                                     bias=bq_sb[:, h, 1:2])
            x1 = qb_lo[:].rearrange("p (h f) -> p h f", h=H)
            x2 = qb_hi[:].rearrange("p (h f) -> p h f", h=H)

            def cs_pair(src):
                ap = src[:, sl]
                return cbass.AP(ap.tensor, ap.offset,
                                [list(ap.ap[0]), [0, H], [1, FD]])

            cosp, sinp = cs_pair(cos_sb), cs_pair(sin_sb)
            t1 = sc_pool.tile([64, FD2], f32, tag="rot_t1")
            t2 = sc_pool.tile([64, FD2], f32, tag="rot_t2")
            t3 = sc_pool.tile([64, FD2], f32, tag="rot_t3")
            t4 = sc_pool.tile([64, FD2], f32, tag="rot_t4")
            pv = lambda t: t.rearrange("p (h f) -> p h f", h=H)
            nc.vector.tensor_mul(pv(t1), x1, cosp)
            nc.vector.tensor_mul(pv(t2), x2, sinp)
            nc.vector.tensor_mul(pv(t3), x1, sinp)
            nc.vector.tensor_mul(pv(t4), x2, cosp)
            nc.vector.tensor_sub(qt[0:64, :, :], pv(t1), pv(t2))
            nc.vector.tensor_add(qt[64:128, :, :], pv(t3), pv(t4))

        def proj(qc, st=None):
            """Q/K/V projections for seq chunk qc, with the previous chunk's
            deferred normalization stages (`st`) woven in at the points where
            their inputs are provably ready. K first (its rotary unblocks the
            next attention chunk's diagonal), then Q heads, then V."""
            kv2 = big(f"kv2_{qc}")
            kp, vp = kv2[:, 0:FD], kv2[:, FD:FD2]
            qt = qt_pool.tile([P, H, FD], bf16, tag="qt", name=f"qt_{qc}")
            qp2 = big(f"qp2_{qc}")
            if qc == 0:
                # chunk 0 is DMA-bound: interleave all four projection chains
                # t-major with just-in-time DMAs (~224KB per t step vs ~0.9us
                # of matmuls), so the PE lags the DMA stream by one tile
                # instead of waiting for whole weight tensors
                nc.sync.dma_start(cos_sb[:, 0:FD], cos2[:, 0:FD])
                nc.sync.dma_start(sin_sb[:, 0:FD], sin2[:, 0:FD])
                tiles = [xt_pool.tile([P, FD], bf16, tag="xt", name=f"xt_0_{t}")
                         for t in range(DM_TILES)]
                for t in range(DM_TILES):
                    nc.sync.dma_start(tiles[t][:], xT[ds(t * P, P), ds(0, FD)])
                    nc.sync.dma_start(wk_sb[:, t, :], wk_r[:, t, :])
                    nc.sync.dma_start(wq_sb[:, :, t, :], wq_r[:, :, t, :])
                    nc.sync.dma_start(wv_sb[:, t, :], wv_r[:, t, :])
                    mm = dict(start=(t == 0), stop=(t == DM_TILES - 1))
                    nc.tensor.matmul(kp, wk_sb[:, t, :], tiles[t][:], **mm)
                    for h in range(H):
                        nc.tensor.matmul(qp2[:, ds(h * FD, FD)],
                                         wq_sb[:, h, t, :], tiles[t][:], **mm)
                    nc.tensor.matmul(vp, wv_sb[:, t, :], tiles[t][:], **mm)
                nc.sync.dma_start(cos_sb[:, FD:seq], cos2[:, FD:seq])
                nc.sync.dma_start(sin_sb[:, FD:seq], sin2[:, FD:seq])
                nc.sync.dma_start(wo_sb[:], wo.rearrange("h p m -> p h m"))
                rotary_evac(kp, kt_sb[:, ds(qc * FD, FD)], bk_sb, qc)
                rotary_evac(qp2[:, 0:FD], qt[:, 0, :], bq_sb[:, 0, :], qc)
                rotary_evac(qp2[:, FD:FD2], qt[:, 1, :], bq_sb[:, 1, :], qc)
            else:
                # Q chains first: the DVE rotary window (qp-done .. next att)
                # must fit the whole Q rotary; K's rotary is only needed by
                # the next chunk's *diagonal* score tiles, several iterations
                # into its kt loop.
                tiles = xts.pop(qc)
                if st is not None:
                    st["recips"]()
                for t in range(DM_TILES):
                    mm = dict(start=(t == 0), stop=(t == DM_TILES - 1))
                    for h in range(H):
                        nc.tensor.matmul(qp2[:, ds(h * FD, FD)],
                                         wq_sb[:, h, t, :], tiles[t][:], **mm)
                if st is not None:
                    st["bcasts"]()
                    st["zmuls"]()
                rotary_q_pair(qp2, qt, qc)
                for t in range(DM_TILES):
                    nc.tensor.matmul(kp, wk_sb[:, t, :], tiles[t][:],
                                     start=(t == 0), stop=(t == DM_TILES - 1))
                rotary_evac(kp, kt_sb[:, ds(qc * FD, FD)], bk_sb, qc)
                for t in range(DM_TILES):
                    nc.tensor.matmul(vp, wv_sb[:, t, :], tiles[t][:],
                                     start=(t == 0), stop=(t == DM_TILES - 1))

            # V: bias add on Act (idle between exp chains), then transpose to
            # natural [k, d] via the DMA engine's XBAR (2-byte dtypes only) —
            # frees the PE transposes and DVE copies entirely
            vt = sc_pool.tile([P, FD], bf16, tag="vt", name=f"vt_{qc}")
            nc.scalar.activation(vt[:], vp, AF.Identity, bias=bv_sb[:, 0:1])
            nc.sync.dma_start_transpose(v_sb[:, ds(qc * KB, KB), :], vt[:])
            return qt

        def attention(qc, qt, pending_evacs):
            """Causal attention for q chunk qc. The previous chunk's outproj
            evacuations (DVE) are emitted one per kt iteration so they never
            queue ahead of this chunk's diagonal masks on the DVE. Returns
            (z2, stages): z2 the normalized [128, 2*FD] bf16 z tiles
            (head-major halves), stages the deferred normalization
            emitters."""
            KT = 4 * qc + 4
            zt2 = big(f"zt2_{qc}")
            den2 = big(f"den2_{qc}")
            pend = None
            for kt in range(KT):
                if pending_evacs:
                    pending_evacs.pop(0)()
                o = max(0, kt * P - qc * FD)
                n = FD - o
                st2 = big(f"st2_{qc}_{kt}")
                for h in range(H):
                    nc.tensor.matmul(st2[:, ds(h * FD + o, n)],
                                     kt_sb[:, ds(kt * P, P)], qt[:, h, o:FD],
                                     start=True, stop=True)
                e2 = e_pool.tile([P, FD2], bf16, tag="e", name=f"e_{qc}_{kt}")
                ev = e2.rearrange("p (h f) -> p h f", h=H)
                sv = st2.rearrange("p (h f) -> p h f", h=H)
                nc.scalar.activation(ev[:, :, o:FD], sv[:, :, o:FD], AF.Exp,
                                     scale=1.0 / ATTN_SCALE)
                if kt >= 4 * qc:  # diagonal 128-block: causal mask inside
                    for h in range(H):
                        nc.vector.tensor_mul(e2[:, ds(h * FD + o, P)],
                                             e2[:, ds(h * FD + o, P)], mask_sb[:])
                if pend is not None:
                    pkt, pe2, po, pn = pend
                    acc = dict(start=(pkt == 0), stop=(pkt == KT - 1))
                    for h in range(H):
                        nc.tensor.matmul(zt2[:, ds(h * FD + po, pn)],
                                         v_sb[:, pkt, :],
                                         pe2[:, ds(h * FD + po, pn)], **acc)
                        nc.tensor.matmul(den2[0:1, ds(h * FD + po, pn)],
                                         ones_sb[:, 0:1],
                                         pe2[:, ds(h * FD + po, pn)], **acc)
                pend = (kt, e2, o, n)
            pkt, pe2, po, pn = pend
            acc = dict(start=(pkt == 0), stop=(pkt == KT - 1))
            for h in range(H):
                nc.tensor.matmul(zt2[:, ds(h * FD + po, pn)], v_sb[:, pkt, :],
                                 pe2[:, ds(h * FD + po, pn)], **acc)
                nc.tensor.matmul(den2[0:1, ds(h * FD + po, pn)],
                                 ones_sb[:, 0:1],
                                 pe2[:, ds(h * FD + po, pn)], **acc)
            while pending_evacs:
                pending_evacs.pop(0)()

            z2 = sc_pool.tile([P, FD2], bf16, tag="z", bufs=2, name=f"z2_{qc}")
            box = {}

            def recips():  # DVE: 1/den for both heads in one sweep
                rf2 = sc_pool.tile([1, FD2], f32, tag="rf", name=f"rf2_{qc}")
                nc.vector.reciprocal_approx_fast(rf2[:], den2[0:1, :])
                rr2 = sc_pool.tile([1, FD2], f32r, tag="rr", name=f"rr2_{qc}")
                nc.vector.tensor_scalar_mul(rr2[:], rf2[:], 1.0)
                box["rr2"] = rr2

            def bcasts():  # PE: broadcast 1/den into den2; Act: copy out
                rr2 = box["rr2"]
                for h in range(H):
                    nc.tensor.matmul(den2[:, ds(h * FD, FD)], onesr_sb[:],
                                     rr2[0:1, ds(h * FD, FD)],
                                     start=True, stop=True)
                rden2 = sc_pool.tile([P, FD2], f32, tag="rden",
                                     name=f"rden2_{qc}")
                nc.scalar.copy(rden2[:], den2[:])
                box["rden2"] = rden2

            def zmuls():  # DVE: z = zt * (1/den), bf16 out
                nc.vector.tensor_mul(z2[:], zt2[:], box["rden2"][:])

            return z2, {"recips": recips, "bcasts": bcasts, "zmuls": zmuls}

        def outproj(qc, z2):
            """Emits the 8 projection chains; returns the evacuation
            emitters (DVE copy + DMA) for the caller to weave into the next
            attention block."""
            evacs = []
            for sub in range(KB):
                for mp in range(MC // 2):
                    op2 = big(f"op2_{qc}_{sub}_{mp}")
                    for half in range(2):
                        mc = 2 * mp + half
                        for h in range(H):
                            nc.tensor.matmul(
                                op2[:, ds(half * FD, FD)],
                                z2[:, ds(h * FD + sub * P, P)],
                                wo_sb[:, h, ds(mc * FD, FD)],
                                start=(h == 0), stop=(h == H - 1))

                    def evac(qc=qc, sub=sub, mp=mp, op2=op2):
                        ot2 = sc_pool.tile([P, FD2], bf16, tag="ot", bufs=3,
                                           name=f"ot_{qc}_{sub}_{mp}")
                        nc.vector.tensor_copy(ot2[:], op2[:])
                        nc.sync.dma_start(
                            out[ds(qc * FD + sub * P, P), ds(mp * FD2, FD2)],
                            ot2[:])

                    evacs.append(evac)
            return evacs

        qts = {0: proj(0)}
        pend_evacs = []
        for qc in range(QC):
            if qc + 1 < QC:
                prefetch_x(qc + 1)
            z2, stages = attention(qc, qts.pop(qc), pend_evacs)
            if qc + 1 < QC:
                qts[qc + 1] = proj(qc + 1, stages)
            else:
                stages["recips"]()
                stages["bcasts"]()
                stages["zmuls"]()
            pend_evacs = outproj(qc, z2)
        for ev in pend_evacs:
            ev()
    nc.compile()
    return nc


_PERM = None


def _perm():
    global _PERM
    if _PERM is None:
        _PERM = np.concatenate([np.arange(0, D_HEAD, 2), np.arange(1, D_HEAD, 2)])
    return _PERM


def host_inputs(x, W_Q, W_K, W_V, W_O, b_Q, b_K, b_V, core,
                heads_per_core=2):
    """Build the per-core input map (numpy, named as in build_bass)."""
    import ml_dtypes

    bf16 = ml_dtypes.bfloat16
    seq = x.shape[0]
    perm = _perm()
    h0 = core * heads_per_core
    kv = h0 // (N_HEADS // N_KV)
    pairs = D_HEAD // 2
    freqs = 1.0 / ROTARY_BASE ** (np.arange(pairs, dtype=np.float64) / pairs)
    ang = np.outer(np.arange(seq), freqs)  # [seq, 64]
    cos = np.cos(ang).T.astype(np.float32)  # [64, seq]
    sin = np.sin(ang).T.astype(np.float32)
    return {
        "xT": np.ascontiguousarray(np.asarray(x).T.astype(bf16)),
        "wq": np.ascontiguousarray(
            W_Q[h0:h0 + heads_per_core][:, :, perm].astype(bf16)),
        "wk": np.ascontiguousarray(W_K[kv][:, perm].astype(bf16)),
        "wv": np.ascontiguousarray(W_V[kv].astype(bf16)),
        "wo": np.ascontiguousarray(W_O[h0:h0 + heads_per_core].astype(bf16)),
        "bq": np.ascontiguousarray(
            b_Q[h0:h0 + heads_per_core][:, perm]
            .reshape(heads_per_core, 2, 64).transpose(2, 0, 1)
            .astype(np.float32)),
        "bk": np.ascontiguousarray(b_K[kv][perm].reshape(2, 64).T
                                   .astype(np.float32)),
        "bv": np.ascontiguousarray(np.asarray(b_V[kv], np.float32)[:, None]),
        "ident": np.eye(P, dtype=np.float32).astype(bf16),
        "cos2": cos,
        "sin2": sin,
        "maskm": np.triu(np.ones((P, P), dtype=np.float32)).astype(bf16),
        "onesd": np.ones((P, 1), dtype=np.float32).astype(bf16),
        "onesr": np.ones((1, P), dtype=np.float32),
    }


_NC_CACHE = {}


def kernel(x, W_Q, W_K, W_V, W_O, b_Q, b_K, b_V, b_O):
    import sys
    if "/opt/trn_rl_repo" not in sys.path:
        sys.path.insert(0, "/opt/trn_rl_repo")
    from concourse import bass_utils

    x = np.asarray(x, dtype=np.float32)
    key = (x.shape[0], x.shape[1])
    if key not in _NC_CACHE:
        _NC_CACHE[key] = build_bass(seq=x.shape[0], d_model=x.shape[1])
    nc = _NC_CACHE[key]

    in_maps = [
        host_inputs(x, np.asarray(W_Q, np.float32), np.asarray(W_K, np.float32),
                    np.asarray(W_V, np.float32), np.asarray(W_O, np.float32),
                    np.asarray(b_Q, np.float32), np.asarray(b_K, np.float32),
                    np.asarray(b_V, np.float32), core)
        for core in range(N_CORES)
    ]
    res = bass_utils.run_bass_kernel_spmd(nc, in_maps, core_ids=list(range(N_CORES)))
    total = np.zeros((x.shape[0], x.shape[1]), dtype=np.float32)
    for r in res.results:
        total += np.asarray(r["out"], dtype=np.float32)
    total += np.asarray(b_O, np.float32)[None, :]
    return total


# revision 50
# speedup vs baseline: 1.0227x; 1.0227x over previous
"""Trainium2 Bass kernel for causal GQA attention (nn_Attention_83090437308676).

Full shapes: x [4096, 2048], 16 Q heads / 4 KV heads, d_head=128, fp32, causal,
rotary (interleaved pairs, rotary_dim=128), out = attn @ W_O + b_O.

Sharding: tensor-parallel over heads. Core c computes Q-heads {2c, 2c+1} and
KV-head c//2 (duplicated across the pair of cores sharing it), produces the
partial output z_h @ W_O_h summed over its 2 heads; the host sums the 8
partials (bf16 on the wire, fp32 accumulate) and adds b_O.

v4 design notes (fp32r baseline: 624us, v2 bf16: 570us, v3: 564us):
- All matmul operands are bf16 (PSUM accumulation stays fp32).
- PSUM is managed as four 4KB "big slots" ([128, 1024] fp32 = 2 banks).
  The two heads of a chunk share slots half-and-half: st2 (scores),
  zt2 (PV accumulators), den2 (softmax denominators), qp2 (Q proj),
  kv2 (K proj | V proj), op2 (out proj pairs of d_model chunks).
- One paired exp per kt tile: a single Act instruction covers both heads
  via a [128, 2, n] strided view, amortizing the PSUM access latency and
  halving Act instruction count (Act was co-critical during attention).
- Attention inner loop keeps the one-iteration skew: st matmuls of both
  heads for kt, then PV/den matmuls of kt-1. The PE never waits on exp.
- Softmax normalization is emitted in stages woven into the *next* chunk's
  projection block at points where each engine is provably past the data
  dependency: DVE reciprocals after the K rotary, PE broadcast + Act copy
  after the Q projection chains, DVE z-multiplies after the Q rotary.
- V is projected [d, k], bias-added on DVE (tensor_scalar add), and
  transposed to natural [k, d] by the PE at the end of the projection.
- Startup DMAs are just-in-time per tile (wk/xt pairs first), so the first
  kp matmul waits on ~2 DMAs; the Exp act table is preloaded via a dummy.
- Output partials are evacuated PSUM->SBUF as bf16 on the DVE in [128,1024]
  pairs and DMA'd as bf16 (2KB per partition row).
"""

import numpy as np

SEQ = 4096
D_MODEL = 2048
D_HEAD = 128
N_HEADS = 16
N_KV = 4
N_CORES = 8
ROTARY_BASE = 10000.0
ATTN_SCALE = 11.313708498984761  # sqrt(d_head)

P = 128  # partitions
FD = 512  # matmul moving free dim / PSUM bank width (fp32)
FD2 = 2 * FD


def build_bass(seq=SEQ, d_model=D_MODEL, heads_per_core=2):
    """Emit the per-core Tile kernel. Same program for all cores (SPMD);
    per-core tensors differ only in data."""
    from contextlib import ExitStack

    import concourse.mybir as mybir
    import concourse.tile as tile
    from concourse import bacc
    from concourse.bass import ds

    f32 = mybir.dt.float32
    f32r = mybir.dt.float32r
    bf16 = mybir.dt.bfloat16
    AF = mybir.ActivationFunctionType
    OP = mybir.AluOpType

    H = heads_per_core
    assert H == 2, "big-slot pairing assumes 2 heads per core"
    DM_TILES = d_model // P      # contraction tiles for projections
    QC = seq // FD               # 512-wide seq chunks
    MC = d_model // FD           # 512-wide output-model chunks
    KB = FD // P                 # 128-wide k blocks per chunk

    nc = bacc.Bacc("TRN2", target_bir_lowering=False, debug=False,
                   num_devices=N_CORES)

    xT = nc.dram_tensor("xT", (d_model, seq), bf16, kind="ExternalInput").ap()
    wq = nc.dram_tensor("wq", (H, d_model, D_HEAD), bf16, kind="ExternalInput").ap()
    wk = nc.dram_tensor("wk", (d_model, D_HEAD), bf16, kind="ExternalInput").ap()
    wv = nc.dram_tensor("wv", (d_model, D_HEAD), bf16, kind="ExternalInput").ap()
    wo = nc.dram_tensor("wo", (H, D_HEAD, d_model), bf16, kind="ExternalInput").ap()
    bq = nc.dram_tensor("bq", (64, H, 2), f32, kind="ExternalInput").ap()
    bk = nc.dram_tensor("bk", (64, 2), f32, kind="ExternalInput").ap()
    bv = nc.dram_tensor("bv", (P, 1), f32, kind="ExternalInput").ap()
    ident = nc.dram_tensor("ident", (P, P), bf16, kind="ExternalInput").ap()
    cos2 = nc.dram_tensor("cos2", (64, seq), f32, kind="ExternalInput").ap()
    sin2 = nc.dram_tensor("sin2", (64, seq), f32, kind="ExternalInput").ap()
    maskm = nc.dram_tensor("maskm", (P, P), bf16, kind="ExternalInput").ap()
    onesd = nc.dram_tensor("onesd", (P, 1), bf16, kind="ExternalInput").ap()
    onesr = nc.dram_tensor("onesr", (1, P), f32r, kind="ExternalInput").ap()
    out = nc.dram_tensor("out", (seq, d_model), bf16, kind="ExternalOutput").ap()

    with tile.TileContext(nc) as tc, ExitStack() as ctx:
        const = ctx.enter_context(tc.tile_pool(name="const", bufs=1))
        persist = ctx.enter_context(tc.tile_pool(name="persist", bufs=1))
        xt_pool = ctx.enter_context(tc.tile_pool(name="xt", bufs=32))
        qt_pool = ctx.enter_context(tc.tile_pool(name="qt", bufs=2))
        e_pool = ctx.enter_context(tc.tile_pool(name="e", bufs=4))
        sc_pool = ctx.enter_context(tc.tile_pool(name="sc", bufs=2))
        ps = ctx.enter_context(tc.tile_pool(name="ps", bufs=4, space="PSUM"))

        def big(name):
            return ps.tile([P, FD2], f32, tag="big", name=name)

        # ---- constants / weights resident in SBUF ----
        wq_sb = const.tile([P, H, DM_TILES, D_HEAD], bf16, tag="wq")
        wk_sb = const.tile([P, DM_TILES, D_HEAD], bf16, tag="wk")
        wv_sb = const.tile([P, DM_TILES, D_HEAD], bf16, tag="wv")
        wq_r = wq.rearrange("h (t p) d -> p h t d", p=P)
        wk_r = wk.rearrange("(t p) d -> p t d", p=P)
        wv_r = wv.rearrange("(t p) d -> p t d", p=P)
        mask_sb = const.tile([P, P], bf16, tag="mask")
        nc.sync.dma_start(mask_sb[:], maskm)
        bq_sb = const.tile([64, H, 2], f32, tag="bq")
        nc.sync.dma_start(bq_sb[:], bq)
        bk_sb = const.tile([64, 2], f32, tag="bk")
        nc.sync.dma_start(bk_sb[:], bk)
        bv_sb = const.tile([P, 1], f32, tag="bv")
        nc.sync.dma_start(bv_sb[:], bv)
        id_sb = const.tile([P, P], bf16, tag="id")
        nc.sync.dma_start(id_sb[:], ident)
        ones_sb = const.tile([P, 1], bf16, tag="ones")
        nc.sync.dma_start(ones_sb[:], onesd)
        onesr_sb = const.tile([1, P], f32r, tag="onesr")
        nc.sync.dma_start(onesr_sb[:], onesr)
        cos_sb = const.tile([64, seq], f32, tag="cos")
        sin_sb = const.tile([64, seq], f32, tag="sin")
        wo_sb = const.tile([P, H, d_model], bf16, tag="wo")
        # preload the Exp activation table off the critical path
        warm = const.tile([1, 2], f32, tag="warm")
        nc.scalar.activation(warm[0:1, 0:2], bq_sb[0:1, 0, 0:2], AF.Exp)

        # K^T (rotated) and V (natural [k, d]) for this core's KV head.
        kt_sb = persist.tile([P, seq], bf16, tag="kt")
        v_sb = persist.tile([P, seq // P, P], bf16, tag="v")

        xts = {}  # chunk -> list of resident xT tiles

        def prefetch_x(qc):
            tiles = [xt_pool.tile([P, FD], bf16, tag="xt", name=f"xt_{qc}_{t}")
                     for t in range(DM_TILES)]
            for t in range(DM_TILES):
                nc.sync.dma_start(tiles[t][:], xT[ds(t * P, P), ds(qc * FD, FD)])
            xts[qc] = tiles

        def rotary_k(kp, qc):
            """K rotary into kt_sb[:, qc chunk]. The kp PSUM halves are
            first bias-copied to partitions 0..63 on the Act engine — this
            frees kp's PSUM slot early (the DVE tensor ops otherwise pin it
            until deep into the next phase) and satisfies the SBUF
            same-base-partition rule for the DVE products."""
            sl = ds(qc * FD, FD)
            dst = kt_sb[:, sl]
            kb_lo = sc_pool.tile([64, FD], f32, tag="kb_lo", bufs=1, name=f"kbl_{qc}")
            kb_hi = sc_pool.tile([64, FD], f32, tag="kb_hi", bufs=1, name=f"kbh_{qc}")
            nc.scalar.activation(kb_lo[:], kp[0:64, :], AF.Identity,
                                 bias=bk_sb[:, 0:1])
            nc.scalar.activation(kb_hi[:], kp[64:128, :], AF.Identity,
                                 bias=bk_sb[:, 1:2])
            t1 = sc_pool.tile([64, FD], f32, tag="rot_t1", bufs=1)
            t2 = sc_pool.tile([64, FD], f32, tag="rot_t2", bufs=1)
            t3 = sc_pool.tile([64, FD], f32, tag="rot_t3", bufs=1)
            t4 = sc_pool.tile([64, FD], f32, tag="rot_t4", bufs=1)
            nc.vector.tensor_mul(t1[:], kb_lo[:], cos_sb[:, sl])
            nc.vector.tensor_mul(t2[:], kb_hi[:], sin_sb[:, sl])
            nc.vector.tensor_mul(t3[:], kb_lo[:], sin_sb[:, sl])
            nc.vector.tensor_mul(t4[:], kb_hi[:], cos_sb[:, sl])
            # rot1 = x1 cos - x2 sin ; rot2 = x1 sin + x2 cos
            nc.vector.tensor_sub(dst[0:64, :], t1[:], t2[:])
            nc.vector.tensor_add(dst[64:128, :], t3[:], t4[:])

        def rotary_q_pair(qp2, qt, qc):
            """Both Q heads' rotary in 6 DVE ops: bias on Act per head and
            partition-half (shifting the x2 half down to partitions 0..63 —
            two SBUF DVE inputs must share a base partition), then
            tensor-tensor products over [64, 2, FD] pair views with the
            cos/sin rows broadcast across heads via a stride-0 free dim."""
            import concourse.bass as cbass
            sl = ds(qc * FD, FD)
            qb_lo = sc_pool.tile([64, FD2], f32, tag="qb_lo", bufs=1, name=f"qbl_{qc}")
            qb_hi = sc_pool.tile([64, FD2], f32, tag="qb_hi", bufs=1, name=f"qbh_{qc}")
            for h in range(H):
                nc.scalar.activation(qb_lo[:, ds(h * FD, FD)],
                                     qp2[0:64, ds(h * FD, FD)], AF.Identity,
                                     bias=bq_sb[:, h, 0:1])
                nc.scalar.activation(qb_hi[:, ds(h * FD, FD)],
                                     qp2[64:128, ds(h * FD, FD)], AF.Identity,
                                     bias=bq_sb[:, h, 1:2])
            x1 = qb_lo[:].rearrange("p (h f) -> p h f", h=H)
            x2 = qb_hi[:].rearrange("p (h f) -> p h f", h=H)

            def cs_pair(src):
                ap = src[:, sl]
                return cbass.AP(ap.tensor, ap.offset,
                                [list(ap.ap[0]), [0, H], [1, FD]])

            cosp, sinp = cs_pair(cos_sb), cs_pair(sin_sb)
            t1 = sc_pool.tile([64, FD2], f32, tag="rot_t1", bufs=1)
            t2 = sc_pool.tile([64, FD2], f32, tag="rot_t2", bufs=1)
            t3 = sc_pool.tile([64, FD2], f32, tag="rot_t3", bufs=1)
            t4 = sc_pool.tile([64, FD2], f32, tag="rot_t4", bufs=1)
            pv = lambda t: t.rearrange("p (h f) -> p h f", h=H)
            nc.vector.tensor_mul(pv(t1), x1, cosp)
            nc.vector.tensor_mul(pv(t2), x2, sinp)
            nc.vector.tensor_mul(pv(t3), x1, sinp)
            nc.vector.tensor_mul(pv(t4), x2, cosp)
            nc.vector.tensor_sub(qt[0:64, :, :], pv(t1), pv(t2))
            nc.vector.tensor_add(qt[64:128, :, :], pv(t3), pv(t4))

        def proj(qc, st=None):
            """Q/K/V projections for seq chunk qc, with the previous chunk's
            deferred normalization stages (`st`) woven in at the points where
            their inputs are provably ready. K first (its rotary unblocks the
            next attention chunk's diagonal), then Q heads, then V."""
            kv2 = big(f"kv2_{qc}")
            kp, vp = kv2[:, 0:FD], kv2[:, FD:FD2]
            qt = qt_pool.tile([P, H, FD], bf16, tag="qt", name=f"qt_{qc}")
            qp2 = big(f"qp2_{qc}")
            if qc == 0:
                # chunk 0 is DMA-bound: interleave all four projection chains
                # t-major with just-in-time DMAs (~224KB per t step vs ~0.9us
                # of matmuls), so the PE lags the DMA stream by one tile
                # instead of waiting for whole weight tensors
                nc.sync.dma_start(cos_sb[:, 0:FD], cos2[:, 0:FD])
                nc.sync.dma_start(sin_sb[:, 0:FD], sin2[:, 0:FD])
                tiles = [xt_pool.tile([P, FD], bf16, tag="xt", name=f"xt_0_{t}")
                         for t in range(DM_TILES)]
                for t in range(DM_TILES):
                    nc.sync.dma_start(tiles[t][:], xT[ds(t * P, P), ds(0, FD)])
                    nc.sync.dma_start(wk_sb[:, t, :], wk_r[:, t, :])
                    nc.sync.dma_start(wq_sb[:, :, t, :], wq_r[:, :, t, :])
                    nc.sync.dma_start(wv_sb[:, t, :], wv_r[:, t, :])
                    mm = dict(start=(t == 0), stop=(t == DM_TILES - 1))
                    nc.tensor.matmul(kp, wk_sb[:, t, :], tiles[t][:], **mm)
                    for h in range(H):
                        nc.tensor.matmul(qp2[:, ds(h * FD, FD)],
                                         wq_sb[:, h, t, :], tiles[t][:], **mm)
                    nc.tensor.matmul(vp, wv_sb[:, t, :], tiles[t][:], **mm)
                nc.sync.dma_start(cos_sb[:, FD:seq], cos2[:, FD:seq])
                nc.sync.dma_start(sin_sb[:, FD:seq], sin2[:, FD:seq])
                nc.sync.dma_start(wo_sb[:], wo.rearrange("h p m -> p h m"))
                rotary_k(kp, qc)
                rotary_q_pair(qp2, qt, qc)
            else:
                # Q chains first: the DVE rotary window (qp-done .. next att)
                # must fit the whole Q rotary; K's rotary is only needed by
                # the next chunk's *diagonal* score tiles, several iterations
                # into its kt loop.
                tiles = xts.pop(qc)
                if st is not None:
                    st["recips"]()
                for t in range(DM_TILES):
                    mm = dict(start=(t == 0), stop=(t == DM_TILES - 1))
                    for h in range(H):
                        nc.tensor.matmul(qp2[:, ds(h * FD, FD)],
                                         wq_sb[:, h, t, :], tiles[t][:], **mm)
                if st is not None:
                    st["bcasts"]()
                    st["zmuls"]()
                rotary_q_pair(qp2, qt, qc)
                for t in range(DM_TILES):
                    nc.tensor.matmul(kp, wk_sb[:, t, :], tiles[t][:],
                                     start=(t == 0), stop=(t == DM_TILES - 1))
                rotary_k(kp, qc)
                for t in range(DM_TILES):
                    nc.tensor.matmul(vp, wv_sb[:, t, :], tiles[t][:],
                                     start=(t == 0), stop=(t == DM_TILES - 1))

            # V: bias add on Act (idle between exp chains), then transpose to
            # natural [k, d] via the DMA engine's XBAR (2-byte dtypes only) —
            # frees the PE transposes and DVE copies entirely
            vt = sc_pool.tile([P, FD], bf16, tag="vt", name=f"vt_{qc}")
            nc.scalar.activation(vt[:], vp, AF.Identity, bias=bv_sb[:, 0:1])
            nc.sync.dma_start_transpose(v_sb[:, ds(qc * KB, KB), :], vt[:])
            return qt

        def attention(qc, qt, pending_evacs):
            """Causal attention for q chunk qc. The previous chunk's outproj
            evacuations (DVE) are emitted one per kt iteration so they never
            queue ahead of this chunk's diagonal masks on the DVE. Returns
            (z2, stages): z2 the normalized [128, 2*FD] bf16 z tiles
            (head-major halves), stages the deferred normalization
            emitters."""
            KT = 4 * qc + 4
            zt2 = big(f"zt2_{qc}")
            den2 = big(f"den2_{qc}")
            pend = None
            for kt in range(KT):
                o = max(0, kt * P - qc * FD)
                n = FD - o
                st2 = big(f"st2_{qc}_{kt}")
                for h in range(H):
                    nc.tensor.matmul(st2[:, ds(h * FD + o, n)],
                                     kt_sb[:, ds(kt * P, P)], qt[:, h, o:FD],
                                     start=True, stop=True)
                e2 = e_pool.tile([P, FD2], bf16, tag="e", name=f"e_{qc}_{kt}")
                ev = e2.rearrange("p (h f) -> p h f", h=H)
                sv = st2.rearrange("p (h f) -> p h f", h=H)
                nc.scalar.activation(ev[:, :, o:FD], sv[:, :, o:FD], AF.Exp,
                                     scale=1.0 / ATTN_SCALE)
                if kt >= 4 * qc:  # diagonal 128-block: causal mask inside
                    for h in range(H):
                        nc.vector.tensor_mul(e2[:, ds(h * FD + o, P)],
                                             e2[:, ds(h * FD + o, P)], mask_sb[:])
                if pend is not None:
                    pkt, pe2, po, pn = pend
                    acc = dict(start=(pkt == 0), stop=(pkt == KT - 1))
                    for h in range(H):
                        nc.tensor.matmul(zt2[:, ds(h * FD + po, pn)],
                                         v_sb[:, pkt, :],
                                         pe2[:, ds(h * FD + po, pn)], **acc)
                        nc.tensor.matmul(den2[0:1, ds(h * FD + po, pn)],
                                         ones_sb[:, 0:1],
                                         pe2[:, ds(h * FD + po, pn)], **acc)
                pend = (kt, e2, o, n)
            pkt, pe2, po, pn = pend
            acc = dict(start=(pkt == 0), stop=(pkt == KT - 1))
            for h in range(H):
                nc.tensor.matmul(zt2[:, ds(h * FD + po, pn)], v_sb[:, pkt, :],
                                 pe2[:, ds(h * FD + po, pn)], **acc)
                nc.tensor.matmul(den2[0:1, ds(h * FD + po, pn)],
                                 ones_sb[:, 0:1],
                                 pe2[:, ds(h * FD + po, pn)], **acc)
            while pending_evacs:
                pending_evacs.pop(0)()

            z2 = sc_pool.tile([P, FD2], bf16, tag="z", bufs=2, name=f"z2_{qc}")
            box = {}

            def recips():  # DVE: 1/den for both heads in one sweep
                rf2 = sc_pool.tile([1, FD2], f32, tag="rf", bufs=1, name=f"rf2_{qc}")
                nc.vector.reciprocal_approx_fast(rf2[:], den2[0:1, :])
                rr2 = sc_pool.tile([1, FD2], f32r, tag="rr", name=f"rr2_{qc}")
                nc.vector.tensor_scalar_mul(rr2[:], rf2[:], 1.0)
                box["rr2"] = rr2

            def bcasts():  # PE: broadcast 1/den into den2; Act: copy out
                rr2 = box["rr2"]
                for h in range(H):
                    nc.tensor.matmul(den2[:, ds(h * FD, FD)], onesr_sb[:],
                                     rr2[0:1, ds(h * FD, FD)],
                                     start=True, stop=True)
                rden2 = sc_pool.tile([P, FD2], f32, tag="rden",
                                     name=f"rden2_{qc}")
                nc.scalar.copy(rden2[:], den2[:])
                box["rden2"] = rden2

            def zmuls():  # DVE: z = zt * (1/den), bf16 out
                nc.vector.tensor_mul(z2[:], zt2[:], box["rden2"][:])

            return z2, {"recips": recips, "bcasts": bcasts, "zmuls": zmuls}

        def outproj(qc, z2):
            """Emits the 8 projection chains; returns the evacuation
            emitters (DVE copy + DMA) for the caller to weave into the next
            attention block."""
            evacs = []
            for sub in range(KB):
                for mp in range(MC // 2):
                    op2 = big(f"op2_{qc}_{sub}_{mp}")
                    for half in range(2):
                        mc = 2 * mp + half
                        for h in range(H):
                            nc.tensor.matmul(
                                op2[:, ds(half * FD, FD)],
                                z2[:, ds(h * FD + sub * P, P)],
                                wo_sb[:, h, ds(mc * FD, FD)],
                                start=(h == 0), stop=(h == H - 1))

                    def evac(qc=qc, sub=sub, mp=mp, op2=op2):
                        ot2 = sc_pool.tile([P, FD2], bf16, tag="ot", bufs=3,
                                           name=f"ot_{qc}_{sub}_{mp}")
                        nc.vector.tensor_copy(ot2[:], op2[:])
                        nc.sync.dma_start(
                            out[ds(qc * FD + sub * P, P), ds(mp * FD2, FD2)],
                            ot2[:])

                    evacs.append(evac)
            return evacs

        qts = {0: proj(0)}
        pend_evacs = []
        for qc in range(QC):
            if qc + 1 < QC:
                prefetch_x(qc + 1)
            z2, stages = attention(qc, qts.pop(qc), pend_evacs)
            if qc + 1 < QC:
                qts[qc + 1] = proj(qc + 1, stages)
            else:
                stages["recips"]()
                stages["bcasts"]()
                stages["zmuls"]()
            pend_evacs = outproj(qc, z2)
        for ev in pend_evacs:
            ev()
    nc.compile()
    return nc


_PERM = None


def _perm():
    global _PERM
    if _PERM is None:
        _PERM = np.concatenate([np.arange(0, D_HEAD, 2), np.arange(1, D_HEAD, 2)])
    return _PERM


def host_inputs(x, W_Q, W_K, W_V, W_O, b_Q, b_K, b_V, core,
                heads_per_core=2):
    """Build the per-core input map (numpy, named as in build_bass)."""
    import ml_dtypes

    bf16 = ml_dtypes.bfloat16
    seq = x.shape[0]
    perm = _perm()
    h0 = core * heads_per_core
    kv = h0 // (N_HEADS // N_KV)
    pairs = D_HEAD // 2
    freqs = 1.0 / ROTARY_BASE ** (np.arange(pairs, dtype=np.float64) / pairs)
    ang = np.outer(np.arange(seq), freqs)  # [seq, 64]
    cos = np.cos(ang).T.astype(np.float32)  # [64, seq]
    sin = np.sin(ang).T.astype(np.float32)
    return {
        "xT": np.ascontiguousarray(np.asarray(x).T.astype(bf16)),
        "wq": np.ascontiguousarray(
            W_Q[h0:h0 + heads_per_core][:, :, perm].astype(bf16)),
        "wk": np.ascontiguousarray(W_K[kv][:, perm].astype(bf16)),
        "wv": np.ascontiguousarray(W_V[kv].astype(bf16)),
        "wo": np.ascontiguousarray(W_O[h0:h0 + heads_per_core].astype(bf16)),
        "bq": np.ascontiguousarray(
            b_Q[h0:h0 + heads_per_core][:, perm]
            .reshape(heads_per_core, 2, 64).transpose(2, 0, 1)
            .astype(np.float32)),
        "bk": np.ascontiguousarray(b_K[kv][perm].reshape(2, 64).T
                                   .astype(np.float32)),
        "bv": np.ascontiguousarray(np.asarray(b_V[kv], np.float32)[:, None]),
        "ident": np.eye(P, dtype=np.float32).astype(bf16),
        "cos2": cos,
        "sin2": sin,
        "maskm": np.triu(np.ones((P, P), dtype=np.float32)).astype(bf16),
        "onesd": np.ones((P, 1), dtype=np.float32).astype(bf16),
        "onesr": np.ones((1, P), dtype=np.float32),
    }


_NC_CACHE = {}


def kernel(x, W_Q, W_K, W_V, W_O, b_Q, b_K, b_V, b_O):
    import sys
    if "/opt/trn_rl_repo" not in sys.path:
        sys.path.insert(0, "/opt/trn_rl_repo")
    from concourse import bass_utils

    x = np.asarray(x, dtype=np.float32)
    key = (x.shape[0], x.shape[1])
    if key not in _NC_CACHE:
        _NC_CACHE[key] = build_bass(seq=x.shape[0], d_model=x.shape[1])
    nc = _NC_CACHE[key]

    in_maps = [
        host_inputs(x, np.asarray(W_Q, np.float32), np.asarray(W_K, np.float32),
                    np.asarray(W_V, np.float32), np.asarray(W_O, np.float32),
                    np.asarray(b_Q, np.float32), np.asarray(b_K, np.float32),
                    np.asarray(b_V, np.float32), core)
        for core in range(N_CORES)
    ]
    res = bass_utils.run_bass_kernel_spmd(nc, in_maps, core_ids=list(range(N_CORES)))
    total = np.zeros((x.shape[0], x.shape[1]), dtype=np.float32)
    for r in res.results:
        total += np.asarray(r["out"], dtype=np.float32)
    total += np.asarray(b_O, np.float32)[None, :]
    return total
